# revision 4
# baseline (speedup 1.0000x reference)
"""Trainium2 Bass kernel for nn_AttentionBlock (B=8, S=2048, D=512).

Sharding: data-parallel over batch B across the 8 NeuronCores (attention is
per-sequence, weights replicated). Each core runs the full attention block on
its own [S, D] slice; no collectives.

Per-core dataflow (all layouts chosen so softmax/LayerNorm reductions run
along the free dimension and the softmax matrix never needs a transpose):
  xT   = x^T (PE transpose)                                [D, S]  fp32r
  qT   = Wq^T @ xT, kT = Wk^T @ xT                         [D, S]  bf16
  v    = x @ Wv                                            [S, D]  bf16
  sT   = kT^T-tiles @ qT-chunks -> scores^T                [T, Sc] psum fp32
  eT   = exp(sT / sqrt(D))                                 [T, Sc] bf16
  attU = eT^T-tiles @ v  (+ ones-matmul row sums)          [Sc, D] psum fp32
  onx  = LN0(attU / sums + x)                              [S, D]  fp32
  h1   = LN1(gelu(onx + onx @ W1)), h2 = LN2(gelu(onx + h1 @ W2))
  out  = h2 @ W3
Projection/FFN matmuls run in fp32r (full PE rate, ~1e-4 rounding);
attention operands are bf16 (attention output is small vs the residual, so
bf16 rounding there is ~5e-4 absolute on a unit-scale signal).
"""

import numpy as np
from contextlib import ExitStack

import concourse.bass as bass
import concourse.tile as tile
from concourse import bacc, mybir
from concourse.masks import make_identity
from concourse.bass_utils import run_bass_kernel_spmd

# Problem constants (hardcoded per harness contract).
B, S, D = 8, 2048, 512
P = 128
NB = S // P            # 16 row blocks
KT = D // P            # 4 contraction tiles
SCW = 512              # attention s-chunk width
NSC = S // SCW         # 4 chunks
JB = SCW // P          # 4 s-blocks per chunk
EPS = 1e-5
SMSCALE = 1.0 / float(np.sqrt(D))   # BETA=1.0

F32 = mybir.dt.float32
F32R = mybir.dt.float32r
BF16 = mybir.dt.bfloat16
AF = mybir.ActivationFunctionType
ALU = mybir.AluOpType

WNAMES = ["Wq", "Wk", "Wv", "W1", "W2", "W3"]
BNAMES = ["bq", "bk", "bv", "b1", "b2", "b3"]
LNNAMES = ["ln0_g", "ln0_b", "ln1_g", "ln1_b", "ln2_g", "ln2_b"]


def _bcast_ap(ap, parts):
    """[D] dram AP -> [parts, D] AP broadcast along partitions."""
    return bass.AP(tensor=ap.tensor, offset=ap.offset, ap=[[0, parts]] + ap.ap)


def _emit(ctx, tc, cfg):
    nc = tc.nc

    # ---- DRAM I/O ----
    x = nc.dram_tensor("x", [S, D], F32, kind="ExternalInput").ap()
    w_ap = {n: nc.dram_tensor(n, [D, D], F32, kind="ExternalInput").ap()
            for n in WNAMES}
    b_ap = {n: nc.dram_tensor(n, [D], F32, kind="ExternalInput").ap()
            for n in BNAMES if n in cfg["nz_bias"]}
    ln_ap = {n: nc.dram_tensor(n, [D], F32, kind="ExternalInput").ap()
             for n in LNNAMES if n in cfg["nz_ln"]}
    out = nc.dram_tensor("out", [S, D], F32, kind="ExternalOutput").ap()

    # ---- pools ----
    consts = ctx.enter_context(tc.tile_pool(name="consts", bufs=1))
    wpool = ctx.enter_context(tc.tile_pool(name="wpool", bufs=3))
    bigp = ctx.enter_context(tc.tile_pool(name="big", bufs=1))
    xep = ctx.enter_context(tc.tile_pool(name="xe", bufs=2))
    xld = ctx.enter_context(tc.tile_pool(name="xld", bufs=3))
    work = ctx.enter_context(tc.tile_pool(name="work", bufs=6))
    ttp = ctx.enter_context(tc.tile_pool(name="ttp", bufs=4))
    small = ctx.enter_context(tc.tile_pool(name="small", bufs=4))
    psb = ctx.enter_context(tc.tile_pool(name="psb", bufs=2, space="PSUM"))
    psa = ctx.enter_context(tc.tile_pool(name="psa", bufs=2, space="PSUM"))
    pss = ctx.enter_context(tc.tile_pool(name="pss", bufs=2, space="PSUM"))
    pst = ctx.enter_context(tc.tile_pool(name="pst", bufs=2, space="PSUM"))

    # ---- constants ----
    ident = consts.tile([P, P], F32)
    make_identity(nc, ident[:])
    ones_f = consts.tile([P, 2], F32)
    nc.vector.memset(ones_f[:], 1.0)
    ones_b = consts.tile([P, 2], BF16)
    nc.vector.tensor_copy(ones_b[:], ones_f[:])
    eps_t = consts.tile([P, 1], F32)
    nc.vector.memset(eps_t[:], EPS)

    # per-partition biases for qT/kT layouts ([dout] on partitions)
    pp_bias = {}
    for n in ("bq", "bk"):
        if n in b_ap:
            t = consts.tile([P, KT], F32, tag=f"pp_{n}")
            nc.sync.dma_start(t[:], b_ap[n].rearrange("(kt p) -> p kt", p=P))
            pp_bias[n] = t
    # broadcast-along-partition biases/gains ([d] on free dim)
    bc_tile = {}
    for n in ("bv", "b1", "b2", "b3"):
        if n in b_ap:
            t = consts.tile([P, D], F32, tag=f"bc_{n}")
            nc.sync.dma_start(t[:], _bcast_ap(b_ap[n], P))
            bc_tile[n] = t
    for n in LNNAMES:
        if n in ln_ap:
            t = consts.tile([P, D], F32, tag=f"bc_{n}")
            nc.sync.dma_start(t[:], _bcast_ap(ln_ap[n], P))
            bc_tile[n] = t

    # ---- persistent per-sequence tensors ----
    qT = bigp.tile([P, KT, S], BF16, tag="qT")
    kTt = bigp.tile([P, KT, S], BF16, tag="kT")
    vt = bigp.tile([P, NB, D], BF16, tag="v")
    onx = bigp.tile([P, NB, D], F32, tag="onx")

    def load_w(name):
        wt = wpool.tile([P, KT, D], F32R, tag="w")
        nc.sync.dma_start(wt[:], w_ap[name]
                          .rearrange("(kt p) d -> p kt d", p=P).bitcast(F32R))
        return wt

    def layer_norm(src, dst, g_name, b_name):
        """dst = LN(src[:, :]) * g + b along the free dim (D=512)."""
        st = small.tile([P, 6], F32, tag="bst")
        nc.vector.bn_stats(st[:], src)
        mv = small.tile([P, 2], F32, tag="mv")
        nc.vector.bn_aggr(mv[:], st[:])
        std = small.tile([P, 1], F32, tag="std")
        nc.scalar.activation(std[:], mv[:, 1:2], AF.Sqrt, bias=eps_t[:])
        rstd = small.tile([P, 1], F32, tag="rstd")
        nc.vector.reciprocal(rstd[:], std[:])
        negm = small.tile([P, 1], F32, tag="negm")
        nc.vector.tensor_scalar_mul(negm[:], mv[:, 0:1], -1.0)
        g = bc_tile.get(g_name)
        b = bc_tile.get(b_name)
        if g is None and b is None:
            nc.vector.tensor_scalar(dst, src, negm[:], rstd[:],
                                    op0=ALU.add, op1=ALU.mult)
        else:
            xn = work.tile([P, D], F32, tag="work")
            nc.vector.tensor_scalar(xn[:], src, negm[:], rstd[:],
                                    op0=ALU.add, op1=ALU.mult)
            if g is not None and b is not None:
                t2 = work.tile([P, D], F32, tag="work")
                nc.vector.tensor_mul(t2[:], xn[:], g[:])
                nc.vector.tensor_add(dst, t2[:], b[:])
            elif g is not None:
                nc.vector.tensor_mul(dst, xn[:], g[:])
            else:
                nc.vector.tensor_add(dst, xn[:], b[:])

    # ================= Phase 1: x -> xT, projections =================
    wq = load_w("Wq")
    wk = load_w("Wk")
    wv = load_w("Wv")
    for sc in range(NSC):
        xT_c = xep.tile([P, KT, SCW], F32R, tag="xe")
        for j in range(JB):
            n = sc * JB + j
            xb = xld.tile([P, D], F32, tag="xld")
            nc.sync.dma_start(xb[:], x[n * P:(n + 1) * P, :])
            for dt in range(KT):
                pt = pst.tile([P, P], F32, tag="pt")
                nc.tensor.transpose(pt[:], xb[:, dt * P:(dt + 1) * P], ident[:])
                nc.vector.tensor_copy(xT_c[:, dt, j * P:(j + 1) * P], pt[:])
        # v blocks of this chunk: v[s,:] = x @ Wv  (lhsT = xT columns)
        for j in range(JB):
            n = sc * JB + j
            pm = psb.tile([P, D], F32, tag="mm")
            for kt in range(KT):
                nc.tensor.matmul(pm[:], xT_c[:, kt, j * P:(j + 1) * P],
                                 wv[:, kt, :], start=(kt == 0),
                                 stop=(kt == KT - 1))
            if "bv" in bc_tile:
                nc.vector.tensor_add(vt[:, n, :], pm[:], bc_tile["bv"][:])
            else:
                nc.vector.tensor_copy(vt[:, n, :], pm[:])
        # qT / kT columns of this chunk: qT = Wq^T @ xT
        for dt in range(KT):
            pmq = psb.tile([P, SCW], F32, tag="mm")
            for kt in range(KT):
                nc.tensor.matmul(pmq[:], wq[:, kt, dt * P:(dt + 1) * P],
                                 xT_c[:, kt, :], start=(kt == 0),
                                 stop=(kt == KT - 1))
            dst = qT[:, dt, sc * SCW:(sc + 1) * SCW]
            if "bq" in pp_bias:
                nc.scalar.activation(dst, pmq[:], AF.Identity,
                                     bias=pp_bias["bq"][:, dt:dt + 1])
            else:
                nc.scalar.copy(dst, pmq[:])
            pmk = psb.tile([P, SCW], F32, tag="mm")
            for kt in range(KT):
                nc.tensor.matmul(pmk[:], wk[:, kt, dt * P:(dt + 1) * P],
                                 xT_c[:, kt, :], start=(kt == 0),
                                 stop=(kt == KT - 1))
            dst = kTt[:, dt, sc * SCW:(sc + 1) * SCW]
            if "bk" in pp_bias:
                nc.scalar.activation(dst, pmk[:], AF.Identity,
                                     bias=pp_bias["bk"][:, dt:dt + 1])
            else:
                nc.scalar.copy(dst, pmk[:])

    # ================= Phase 2: attention =================
    for sc in range(NSC):
        eT = xep.tile([P, NB, SCW], BF16, tag="xe")
        for tt in range(NB):
            pm = psb.tile([P, SCW], F32, tag="mm")
            for kt in range(KT):
                nc.tensor.matmul(pm[:], kTt[:, kt, tt * P:(tt + 1) * P],
                                 qT[:, kt, sc * SCW:(sc + 1) * SCW],
                                 start=(kt == 0), stop=(kt == KT - 1))
            nc.scalar.activation(eT[:, tt, :], pm[:], AF.Exp, scale=SMSCALE)
        for j in range(JB):
            n = sc * JB + j
            pa = psa.tile([P, D], F32, tag="att")
            psm = pss.tile([P, 2], F32, tag="sm")
            for tt in range(NB):
                nc.tensor.matmul(pa[:], eT[:, tt, j * P:(j + 1) * P],
                                 vt[:, tt, :], start=(tt == 0),
                                 stop=(tt == NB - 1))
                nc.tensor.matmul(psm[:], eT[:, tt, j * P:(j + 1) * P],
                                 ones_b[:], start=(tt == 0),
                                 stop=(tt == NB - 1))
            rcp = small.tile([P, 1], F32, tag="rcp")
            nc.vector.reciprocal(rcp[:], psm[:, 0:1])
            xr = xld.tile([P, D], F32, tag="xld")
            nc.sync.dma_start(xr[:], x[n * P:(n + 1) * P, :])
            att = work.tile([P, D], F32, tag="work")
            nc.vector.scalar_tensor_tensor(att[:], pa[:], rcp[:], xr[:],
                                           op0=ALU.mult, op1=ALU.add)
            layer_norm(att[:], onx[:, n, :], "ln0_g", "ln0_b")

    # ================= Phase 3: FFN =================
    w1 = load_w("W1")
    w2 = load_w("W2")
    w3 = load_w("W3")

    def transpose_block(src):
        """[P(s), D] f32 -> [P(d), KT, P(s)] fp32r."""
        t = ttp.tile([P, KT, P], F32R, tag="tT")
        for dt in range(KT):
            pt = pst.tile([P, P], F32, tag="pt")
            nc.tensor.transpose(pt[:], src[:, dt * P:(dt + 1) * P], ident[:])
            nc.vector.tensor_copy(t[:, dt, :], pt[:])
        return t

    for n in range(NB):
        t1 = transpose_block(onx[:, n, :])
        # h1 = LN1(gelu(onx + onx @ W1 + b1))
        pm = psb.tile([P, D], F32, tag="mm")
        for kt in range(KT):
            nc.tensor.matmul(pm[:], t1[:, kt, :], w1[:, kt, :],
                             start=(kt == 0), stop=(kt == KT - 1))
        pre = work.tile([P, D], F32, tag="work")
        nc.vector.tensor_add(pre[:], pm[:], onx[:, n, :])
        if "b1" in bc_tile:
            nc.vector.tensor_add(pre[:], pre[:], bc_tile["b1"][:])
        gl = work.tile([P, D], F32, tag="work")
        nc.scalar.activation(gl[:], pre[:], AF.Gelu)
        h1 = work.tile([P, D], F32, tag="work")
        layer_norm(gl[:], h1[:], "ln1_g", "ln1_b")
        t2 = transpose_block(h1[:])
        # h2 = LN2(gelu(onx + h1 @ W2 + b2))
        pm2 = psb.tile([P, D], F32, tag="mm")
        for kt in range(KT):
            nc.tensor.matmul(pm2[:], t2[:, kt, :], w2[:, kt, :],
                             start=(kt == 0), stop=(kt == KT - 1))
        pre2 = work.tile([P, D], F32, tag="work")
        nc.vector.tensor_add(pre2[:], pm2[:], onx[:, n, :])
        if "b2" in bc_tile:
            nc.vector.tensor_add(pre2[:], pre2[:], bc_tile["b2"][:])
        gl2 = work.tile([P, D], F32, tag="work")
        nc.scalar.activation(gl2[:], pre2[:], AF.Gelu)
        h2 = work.tile([P, D], F32, tag="work")
        layer_norm(gl2[:], h2[:], "ln2_g", "ln2_b")
        t3 = transpose_block(h2[:])
        # out = h2 @ W3 + b3
        pmo = psb.tile([P, D], F32, tag="mm")
        for kt in range(KT):
            nc.tensor.matmul(pmo[:], t3[:, kt, :], w3[:, kt, :],
                             start=(kt == 0), stop=(kt == KT - 1))
        ot = work.tile([P, D], F32, tag="work")
        if "b3" in bc_tile:
            nc.vector.tensor_add(ot[:], pmo[:], bc_tile["b3"][:])
        else:
            nc.vector.tensor_copy(ot[:], pmo[:])
        nc.sync.dma_start(out[n * P:(n + 1) * P, :], ot[:])


def build_nc(cfg):
    nc = bacc.Bacc("TRN2", target_bir_lowering=False, debug=False)
    with tile.TileContext(nc) as tc:
        with ExitStack() as ctx:
            _emit(ctx, tc, cfg)
    nc.compile()
    return nc


def make_cfg(inputs):
    nz_bias = {n for n in BNAMES if np.any(inputs[n] != 0.0)}
    nz_ln = set()
    for n in LNNAMES:
        trivial = np.all(inputs[n] == (1.0 if n.endswith("_g") else 0.0))
        if not trivial:
            nz_ln.add(n)
    return {"nz_bias": nz_bias, "nz_ln": nz_ln}


def _run(inputs, trace=False, nc=None):
    cfg = make_cfg(inputs)
    if nc is None:
        nc = build_nc(cfg)
    in_maps = []
    for b in range(B):
        m = {"x": np.ascontiguousarray(inputs["x"][b], dtype=np.float32)}
        for n in WNAMES:
            m[n] = np.ascontiguousarray(inputs[n], dtype=np.float32)
        for n in cfg["nz_bias"]:
            m[n] = np.ascontiguousarray(inputs[n], dtype=np.float32)
        for n in cfg["nz_ln"]:
            m[n] = np.ascontiguousarray(inputs[n], dtype=np.float32)
        in_maps.append(m)
    res = run_bass_kernel_spmd(nc, in_maps, core_ids=list(range(B)),
                               trace=trace)
    out = np.stack([res.results[b]["out"] for b in range(B)], axis=0)
    return out.astype(np.float32), res


def kernel(**inputs):
    out, _ = _run(inputs, trace=False)
    return out
